# revision 60
# baseline (speedup 1.0000x reference)
"""Multi-head self-attention Bass kernel for 8 TRN2 NeuronCores.

Problem: B=8, N=1024, C=1024, H=16, D=64, fp32 in/out.
Sharding: data-parallel over batch -- core b computes batch element b
end-to-end; no collectives.  ~267us HW (baseline 365us).

Design:
  - all matmuls bf16 (host casts x/w; rel err ~5.5e-3 < 2e-2 gate)
  - PE HAM clock management: dependency-free warm-up matmuls at t=0,
    then one continuous PE stream (prologue -> ACT-gated pair phase ->
    projection tail) so the clock stays at 2.4 GHz (the PE drops to
    1.2 GHz after any ~3.4us idle gap)
  - scores psum tiles [128,1024] hold BOTH heads of a slab in the free
    dim ([head-even n-half | head-odd n-half]) so one exp ACT releases
    both row-group matmuls together -> the two K=64 scores MMs run
    concurrently in disjoint PE row groups
  - middle phase: per (pair s, m-tile) slot emits 4 scores MMs + 2 exp
    ACTs + 4 AV MMs of pair s-1 (one slot later, so pair boundaries
    don't bubble the ACT queue) + one q/k/v slab-fill unit, keeping the
    ~147us of exp hidden under PE work
  - PSUM: scores 2x[128,1024] (4 banks) + AV acc 2x[65,512] (2) +
    fill/proj 2x[128,512] (2) = 8 banks exactly
  - AV: per (rowlo,nch) accumulate [65,512] over 8 m-tiles; row 64 is
    the softmax denominator via a ones column in the v slabs; divide =
    psum->SBUF copy (releases the bank early), DVE reciprocal, DRAM
    round-trip partition broadcast, DVE multiply; odd heads bounce
    through an SBUF->SBUF DMA to shift partitions 0-63 -> 64-127
  - tail: pair-7 nch0 AV runs inside pair 7 (filler psum banks) so its
    divide lands at phase end; proj n-rows 0-511 start immediately,
    overlapping the nch1 AV + divide; DMA loads are 2D slab transfers
    split across the sync/gpsimd queues, x strictly first
"""

import os
import sys

sys.path.insert(0, "/opt/trn_rl_repo")

import numpy as np

B, N, C = 8, 1024, 1024
H = 16
D = C // H  # 64
SCALE = D ** -0.5  # 0.125
P = 128
CT = C // P  # 8 contraction tiles of 128

_CACHE = {}

LAST_EXEC_NS = None


def _build():
    import concourse.bacc as bacc
    import concourse.tile as tile
    from concourse import mybir

    fp32 = mybir.dt.float32
    bf16 = mybir.dt.bfloat16
    AFT = mybir.ActivationFunctionType

    nc = bacc.Bacc(
        "TRN2",
        target_bir_lowering=False,
        debug=False,
        enable_asserts=False,
        num_devices=8,
    )
    xT = nc.dram_tensor("xT", [C, N], bf16, kind="ExternalInput")
    wqkvT = nc.dram_tensor("wqkvT", [C, 3 * C], bf16, kind="ExternalInput")
    wprojT = nc.dram_tensor("wprojT", [C, C], bf16, kind="ExternalInput")
    bproj = nc.dram_tensor("bproj", [C], fp32, kind="ExternalInput")
    y = nc.dram_tensor("y", [N, C], fp32, kind="ExternalOutput")

    tap = os.environ.get("MHSA_KERNEL_DEBUG_TAP", "")

    with tile.TileContext(nc) as tc:
        with (
            tc.tile_pool(name="consts", bufs=1) as consts,
            tc.tile_pool(name="xp", bufs=1) as xp,
            tc.tile_pool(name="wq", bufs=4) as wqp,
            tc.tile_pool(name="wp2", bufs=2) as wp2,
            tc.tile_pool(name="qk", bufs=16) as qkp,
            tc.tile_pool(name="vp", bufs=8) as vpp,
            tc.tile_pool(name="et", bufs=26) as etp,
            tc.tile_pool(name="pj", bufs=8) as pjp,
            tc.tile_pool(name="sm", bufs=2) as smp,
            tc.tile_pool(name="avs", bufs=3) as avsp,
            tc.tile_pool(name="rb", bufs=2) as rbp,
            tc.tile_pool(name="tm", bufs=1) as tmp_pool,
            tc.tile_pool(name="ot", bufs=3) as otp,
            tc.tile_pool(name="dscr", bufs=8, space="DRAM") as dscr,
            tc.tile_pool(name="ps_sc", bufs=2, space="PSUM") as ps_sc,
            tc.tile_pool(name="ps_av", bufs=2, space="PSUM") as ps_av,
            tc.tile_pool(name="ps_fl", bufs=2, space="PSUM") as ps_fl,
        ):
            # x resident as ONE tile [128, 8*1024]: column block ci holds
            # xT rows [128ci, 128ci+128) (one coalesced DMA, not 8)
            xs = xp.tile([P, CT * N], bf16, name="xs", tag="xs")

            def xsl(ci, lo, hi):
                return xs[:, ci * N + lo : ci * N + hi]
            qts = [qkp.tile([P, N], bf16, name=f"qs{i}", tag="qk") for i in range(CT)]
            kts = [qkp.tile([P, N], bf16, name=f"ks{i}", tag="qk") for i in range(CT)]
            vss = [
                vpp.tile([P, H * 65], bf16, name=f"vs{i}", tag="vs") for i in range(CT)
            ]
            vvs = [v[:].rearrange("p (h e) -> p h e", e=65) for v in vss]
            pjs = [pjp.tile([P, N], bf16, name=f"pj{i}", tag="pj") for i in range(CT)]
            bb = consts.tile([P, C], fp32)

            # ---- ACT table warm-up: tiny exp so the ~2.7us table load
            # happens during the prologue DMA, not at the first real exp.
            junk = smp.tile([1, 16], fp32, name="junk", tag="junk")
            junk2 = smp.tile([1, 16], fp32, name="junk2", tag="junk")
            nc.gpsimd.memset(junk[:], 0.0)
            nc.scalar.activation(junk2[:], junk[:], AFT.Exp, scale=1.0)

            # ---- PE HAM warm-up: ~5us of dependency-free junk matmuls at
            # t=0 so the PE clock is at 2.4 GHz (K=8/8) by the time the
            # DMA-fed prologue matmuls start.  Without this the whole
            # prologue runs at the cold 1.2 GHz clock.
            jw = consts.tile([P, 512], bf16, name="jw")
            nc.gpsimd.memset(jw[:], 0.0)
            jps = ps_fl.tile([1, 512], fp32, name="jfl", tag="fl")
            NJUNK = 30
            for i in range(NJUNK):
                # one accumulation group: MMs pipeline back-to-back with no
                # inter-MM semaphores; K=128 so HAM sees full-row activity
                nc.tensor.matmul(
                    jps[:],
                    lhsT=jw[:, 0:1],
                    rhs=jw[:, :],
                    start=(i == 0),
                    stop=(i == NJUNK - 1),
                )

            # ---- DMAs: plain 2D slab transfers (fast DMA path), spread
            # across the sync and gpsimd queues.  Priority order: x + the
            # q0/k0 weight blocks (prologue critical path), then the v
            # blocks (needed by pair-0 filler), then everything else.
            wtiles = {}

            def walloc():
                wt = wqp.tile([P, CT * 512], bf16, name="wob", tag="wob")
                return wt

            def load_w_chunk(wt, oblk, ci, eng):
                eng.dma_start(
                    wt[:, ci * 512 : (ci + 1) * 512],
                    wqkvT.ap()[
                        ci * P : (ci + 1) * P, oblk * 512 : (oblk + 1) * 512
                    ],
                )

            def load_w(oblk, flip=0):
                wt = walloc()
                for ci in range(CT):
                    eng = nc.sync if (ci + flip) % 2 == 0 else nc.gpsimd
                    load_w_chunk(wt, oblk, ci, eng)
                wtiles[oblk] = wt

            def wsl(oblk, ci, lo, hi):
                return wtiles[oblk][:, ci * 512 + lo : ci * 512 + hi]

            # v-first prologue: x + the v weight blocks have priority,
            # the q0/k0 blocks follow.  (oblk 4,5 buffers get reused by
            # oblk 1,3 once the v slabs are done -- allocation order
            # matters for the pool rotation.)
            wtiles[4] = walloc()
            wtiles[5] = walloc()
            # x strictly first on both queues (everything needs all of x)
            for ci in range(CT):
                eng = nc.sync if ci % 2 == 0 else nc.gpsimd
                eng.dma_start(
                    xs[:, ci * N : (ci + 1) * N],
                    xT.ap()[ci * P : (ci + 1) * P, :],
                )
            for ci in range(CT):
                load_w_chunk(wtiles[4], 4, ci, nc.gpsimd if ci % 2 == 0 else nc.sync)
            for ci in range(CT):
                load_w_chunk(wtiles[5], 5, ci, nc.sync if ci % 2 == 0 else nc.gpsimd)
            nc.gpsimd.dma_start(bb[:], bproj.ap().partition_broadcast(P))
            load_w(0)
            load_w(2, 1)

            # ---- slab emitters (each "unit" = half a slab: 8 MMs + 1 copy)
            def emit_qk_half(kind, s, half):
                oblk = (0 if kind == "q" else 2) + s // 4
                dst = (qts if kind == "q" else kts)[s]
                ps = ps_fl.tile([P, 512], fp32, name="fl", tag="fl")
                for ci in range(CT):
                    nc.tensor.matmul(
                        ps[:],
                        lhsT=wsl(oblk, ci, (s % 4) * P, (s % 4 + 1) * P),
                        rhs=xsl(ci, half * 512, (half + 1) * 512),
                        start=(ci == 0),
                        stop=(ci == CT - 1),
                    )
                nc.vector.tensor_copy(dst[:, half * 512 : (half + 1) * 512], ps[:])

            def emit_v_half(mi, vblk):
                if vblk == 0:
                    nc.gpsimd.memset(vvs[mi][:, :, 64:65], 1.0)
                ps = ps_fl.tile([P, 512], fp32, name="fl", tag="fl")
                for ci in range(CT):
                    nc.tensor.matmul(
                        ps[:],
                        lhsT=xsl(ci, mi * P, (mi + 1) * P),
                        rhs=wsl(4 + vblk, ci, 0, 512),
                        start=(ci == 0),
                        stop=(ci == CT - 1),
                    )
                nc.vector.tensor_copy(
                    vvs[mi][:, vblk * 8 : (vblk + 1) * 8, 0:64],
                    ps[:].rearrange("p (hh d) -> p hh d", d=64),
                )

            # ---- scores + exp for (pair s, m-tile mi)
            # psum tile layout: cols 0-511 = head 2s (rows 0:64 of slab),
            # cols 512-1023 = head 2s+1 -- both for one n-half.
            eta = [[None] * CT for _ in range(CT)]  # [s][mi] -> n 0:512
            etb = [[None] * CT for _ in range(CT)]  # [s][mi] -> n 512:1024

            def emit_scores(s, mi):
                ta = ps_sc.tile([P, N], fp32, name="sca", tag="sc")
                tb = ps_sc.tile([P, N], fp32, name="scb", tag="sc")
                for t, nch in ((ta, 0), (tb, 1)):
                    for rowlo in (0, 64):
                        nc.tensor.matmul(
                            t[:, (rowlo // 64) * 512 : (rowlo // 64) * 512 + 512],
                            lhsT=kts[s][rowlo : rowlo + 64, mi * P : (mi + 1) * P],
                            rhs=qts[s][
                                rowlo : rowlo + 64, nch * 512 : (nch + 1) * 512
                            ],
                            start=True,
                            stop=True,
                        )
                ea = etp.tile([P, N], bf16, name="eta", tag="et")
                eb = etp.tile([P, N], bf16, name="etb", tag="et")
                nc.scalar.activation(ea[:], ta[:], AFT.Exp, scale=SCALE)
                nc.scalar.activation(eb[:], tb[:], AFT.Exp, scale=SCALE)
                eta[s][mi] = ea
                etb[s][mi] = eb

            # ---- AV for pair sp, staged: nch0 over slots 0-3, nch1 4-7.
            accs = {}
            tail_divs = {(7, 1)}

            def emit_av_slot(sp, slot, acc_pool=None):
                nch = slot // 4
                j = slot % 4
                ets = eta if nch == 0 else etb
                if j == 0:
                    pool = acc_pool if acc_pool is not None else ps_av
                    tg = "sc" if acc_pool is not None else "av"
                    for rowlo in (0, 64):
                        accs[(sp, nch, rowlo)] = pool.tile(
                            [65, 512], fp32, name="av", tag=tg
                        )
                for mi_ in (2 * j, 2 * j + 1):
                    for rowlo in (0, 64):
                        nc.tensor.matmul(
                            accs[(sp, nch, rowlo)][:],
                            lhsT=vvs[mi_][:, 2 * sp + rowlo // 64, :],
                            rhs=ets[sp][mi_][
                                :, (rowlo // 64) * 512 : (rowlo // 64) * 512 + 512
                            ],
                            start=(mi_ == 0),
                            stop=(mi_ == 7),
                        )
                if j == 3:
                    emit_div(sp, nch, tail=(acc_pool is not None or (sp, nch) in tail_divs))

            def emit_av7_imm(mi_):
                # pair-7 nch0 AV accumulated inside pair 7 itself, in the
                # filler psum banks (no fills run during pair 7).  Its
                # divide then completes right at phase end, so proj mi0-3
                # can start without waiting for the whole AV(7) tail.
                if mi_ == 0:
                    for rowlo in (0, 64):
                        accs[(7, 0, rowlo)] = ps_fl.tile(
                            [65, 512], fp32, name="av7", tag="fl"
                        )
                for rowlo in (0, 64):
                    nc.tensor.matmul(
                        accs[(7, 0, rowlo)][:],
                        lhsT=vvs[mi_][:, 14 + rowlo // 64, :],
                        rhs=eta[7][mi_][
                            :, (rowlo // 64) * 512 : (rowlo // 64) * 512 + 512
                        ],
                        start=(mi_ == 0),
                        stop=(mi_ == 7),
                    )

            def emit_div(sp, nch, tail=False):
                # copy acc psum -> SBUF first: releases the psum bank after
                # ~0.7us instead of holding it through the whole divide
                # chain (the next AV group's MMs wait on that bank).  The
                # chain is queue-local: DVE (copy+recip+scr-DMA) then
                # gpsimd (broadcast-DMA+mul+shift-DMA) -- one cross-queue
                # hop total.  Tail divides use the then-idle ACT engine
                # for the copy.
                for rowlo in (0, 64):
                    acc = accs.pop((sp, nch, rowlo))
                    av = avsp.tile([65, 512], fp32, name="avc", tag="avc")
                    if tail and rowlo == 64:
                        nc.scalar.copy(av[:], acc[:])
                    else:
                        nc.vector.tensor_copy(av[:], acc[:])
                    # NB: reciprocal_approx_fast must start at partition 0
                    # and write a separate tile (partial-partition slices
                    # and in-place both produce garbage)
                    rcp = smp.tile([65, 512], fp32, name="rcp", tag="rcp")
                    nc.vector.reciprocal_approx_fast(rcp[:], av[:])
                    scr = dscr.tile([1, 512], fp32, name="scr", tag="scr")
                    # tail divides split their DMA hops across queues so the
                    # two rowlo chains run in parallel (sync is weight/y-only
                    # by then; mid-phase it would head-of-line block)
                    deng = nc.sync if (tail and rowlo == 64) else nc.gpsimd
                    deng.dma_start(scr[:], rcp[64:65, :])
                    rb = rbp.tile([64, 512], fp32, name="rb", tag="rb")
                    deng.dma_start(rb[:], scr[0, :].partition_broadcast(64))
                    dst = pjs[sp][
                        rowlo : rowlo + 64, nch * 512 : (nch + 1) * 512
                    ]
                    if rowlo == 0:
                        nc.vector.tensor_mul(dst, av[0:64, :], rb[:])
                    else:
                        tmp = tmp_pool.tile([64, 512], bf16, name="tmp", tag="tmp")
                        meng = nc.gpsimd if tail else nc.vector
                        meng.tensor_mul(tmp[:], av[0:64, :], rb[:])
                        deng2 = nc.sync if tail else nc.gpsimd
                        deng2.dma_start(dst, tmp[:])

            # ---- prologue PE: v0-3 (chasing the x/w DMA), q0,k0,q1,k1;
            # v4-7 go as pair-0 filler (AV(0) during pair 1 needs them).
            run_heads = tap in ("", "pj", "et")
            if run_heads:
                for mi in range(4):
                    for vblk in range(2):
                        emit_v_half(mi, vblk)
                for half in range(2):
                    emit_qk_half("q", 0, half)
                for half in range(2):
                    emit_qk_half("k", 0, half)

                pwts = []

                def load_wproj():
                    for och in range(2):
                        wt = wp2.tile([P, CT * 512], bf16, name="pwt", tag="pwt")
                        for ci in range(CT):
                            nc.sync.dma_start(
                                wt[:, ci * 512 : (ci + 1) * 512],
                                wprojT.ap()[
                                    ci * P : (ci + 1) * P,
                                    och * 512 : (och + 1) * 512,
                                ],
                            )
                        pwts.append(wt)

                # filler units per pair, consumed one per slot until the
                # pair's list is empty.  Deadlines: v*.vblk0 before pair 1
                # (AV(0) heads 0/1), (k,q)(s) before pair s-1 ends,
                # v*.vblk1 before pair 4 ends (AV(4) heads 8/9).
                pair_units = [[] for _ in range(8)]
                # q1/k1 ride as early pair-0 filler (pair 0 only needs
                # q0/k0), interleaved with the v slabs by deadline
                pair_units[0] = [
                    ("v", 4, 0), ("v", 4, 1), ("k", 1, 0), ("k", 1, 1),
                    ("q", 1, 0), ("q", 1, 1), ("v", 5, 0), ("v", 5, 1),
                    ("v", 6, 0), ("v", 6, 1), ("v", 7, 0), ("v", 7, 1),
                ]
                for s in range(1, 7):
                    pair_units[s] = [("k", s + 1, 0), ("k", s + 1, 1),
                                     ("q", s + 1, 0), ("q", s + 1, 1)]

                def pop_unit(s):
                    if pair_units[s]:
                        kind, a, b = pair_units[s].pop(0)
                        if kind == "v":
                            emit_v_half(a, b)
                        else:
                            emit_qk_half(kind, a, b)

                # ---- main ACT-gated phase.  AV for pair s-1 lags one slot
                # behind the scores of pair s so the next pair's scores MMs
                # always precede the AV group that waits on the previous
                # pair's final exp (kills the pair-boundary ACT bubble).
                for s in range(CT):
                    for mi in range(CT):
                        emit_scores(s, mi)
                        if s == 7:
                            # pair 6's exps are long done: run its AV
                            # unshifted so div(6,*) completes in-phase
                            if mi == 0:
                                emit_av_slot(5, 7)
                            emit_av_slot(6, mi)
                        elif s >= 1 and mi >= 1:
                            emit_av_slot(s - 1, mi - 1)
                        elif s >= 2 and mi == 0:
                            emit_av_slot(s - 2, 7)
                        if s == 7 and mi >= 2:
                            emit_av7_imm(mi - 2)
                        # filler budget: pair 0 has no AV -> 1 unit/slot;
                        # later pairs 1 unit every other slot
                        pop_unit(s)
                        if s == 0 and mi < 4:
                            pop_unit(s)   # pair 0 carries 12 units
                        # late weight loads: oblk1/3 reuse the oblk4/5
                        # buffers, so emit only after the last v-slab
                        # filler MMs (end of pair 0) have been emitted.
                        if s == 1 and mi == 0:
                            load_w(1)
                            load_w(3, 1)
                            load_wproj()
                for s in range(8):
                    while pair_units[s]:
                        pop_unit(s)
                        if s == 0 and mi < 4:
                            pop_unit(s)   # pair 0 carries 12 units

                # ---- tail: AV(7) + divides interleaved with projection
                def emit_proj(mi):
                    for och in range(2):
                        ps = ps_fl.tile([P, 512], fp32, name="fl", tag="fl")
                        for ci in range(CT):
                            nc.tensor.matmul(
                                ps[:],
                                lhsT=pjs[ci][:, mi * P : (mi + 1) * P],
                                rhs=pwts[och][:, ci * 512 : (ci + 1) * 512],
                                start=(ci == 0),
                                stop=(ci == CT - 1),
                            )
                        ot = otp.tile([P, 512], fp32, name="ot", tag="ot")
                        nc.vector.tensor_add(
                            ot[:], ps[:], bb[:, och * 512 : (och + 1) * 512]
                        )
                        yeng = nc.sync if (2 * mi + och) % 2 == 0 else nc.scalar
                        yeng.dma_start(
                            y.ap()[mi * P : (mi + 1) * P, och * 512 : (och + 1) * 512],
                            ot[:],
                        )

                # tail: finish the shifted AV(6) + pair-7 AV, overlapping
                # divides and projection.
                emit_av7_imm(6)
                emit_av7_imm(7)
                emit_div(7, 0, tail=True)
                if tap == "":
                    for mi in range(4):
                        emit_proj(mi)        # gated only on nch0 divides
                for slot in range(4, 8):
                    emit_av_slot(7, slot)    # nch1 + div(7,1)
                if tap == "":
                    for mi in range(4, 8):
                        emit_proj(mi)
            else:
                # debug taps for q/k/v only: emit all slabs plainly
                for mi in range(CT):
                    for vblk in range(2):
                        emit_v_half(mi, vblk)
                for s in range(CT):
                    for kind in ("q", "k"):
                        if s >= 4 and (0 if kind == "q" else 2) + 1 not in wtiles:
                            pass
                        for half in range(2):
                            if s == 4 and half == 0 and kind == "q":
                                load_w(1)
                                load_w(3, 1)
                            emit_qk_half(kind, s, half)

            # ---- debug taps
            if tap in ("q", "k"):
                slabs = qts if tap == "q" else kts
                for s in range(CT):
                    ct = otp.tile([P, N], fp32, name="dbgt", tag="dbgt")
                    nc.vector.tensor_copy(ct[:], slabs[s][:])
                    nc.sync.dma_start(y.ap()[s * P : (s + 1) * P, :], ct[:])
            elif tap == "v":
                for mi in range(CT):
                    ct = otp.tile([P, N], fp32, name="dbgt", tag="dbgt")
                    nc.vector.tensor_copy(
                        ct[:].rearrange("p (h d) -> p h d", d=64),
                        vvs[mi][:, :, 0:64],
                    )
                    nc.sync.dma_start(y.ap()[mi * P : (mi + 1) * P, :], ct[:])
            elif tap == "et":
                # dump pair 7's eta tiles (bf16 -> fp32)
                for mi in range(CT):
                    ct = otp.tile([P, N], fp32, name="dbgt", tag="dbgt")
                    nc.vector.tensor_copy(ct[:], eta[7][mi][:])
                    nc.sync.dma_start(y.ap()[mi * P : (mi + 1) * P, :], ct[:])
            elif tap == "pj":
                for s in range(CT):
                    ct = otp.tile([P, N], fp32, name="dbgt", tag="dbgt")
                    nc.vector.tensor_copy(ct[:], pjs[s][:])
                    nc.sync.dma_start(y.ap()[s * P : (s + 1) * P, :], ct[:])

    nc.compile()
    return nc


def kernel(x, w_qkv, w_proj, b_proj):
    global LAST_EXEC_NS
    import ml_dtypes
    from concourse.bass_utils import run_bass_kernel_spmd

    bf = ml_dtypes.bfloat16
    x = np.asarray(x, dtype=np.float32)
    w_qkv = np.asarray(w_qkv, dtype=np.float32)
    w_proj = np.asarray(w_proj, dtype=np.float32)
    b_proj = np.asarray(b_proj, dtype=np.float32)

    if "nc" not in _CACHE:
        _CACHE["nc"] = _build()
    nc = _CACHE["nc"]

    wqkvT = np.ascontiguousarray(w_qkv.astype(bf).T)
    wprojT = np.ascontiguousarray(w_proj.astype(bf).T)
    xb = x.astype(bf)
    in_maps = [
        {
            "xT": np.ascontiguousarray(xb[b].T),
            "wqkvT": wqkvT,
            "wprojT": wprojT,
            "bproj": b_proj,
        }
        for b in range(B)
    ]
    res = run_bass_kernel_spmd(nc, in_maps, core_ids=list(range(B)))
    if res.exec_time_ns is not None:
        LAST_EXEC_NS = res.exec_time_ns
    return np.stack([res.results[b]["y"] for b in range(B)], axis=0)


# revision 61
# speedup vs baseline: 1.0131x; 1.0131x over previous
"""Multi-head self-attention Bass kernel for 8 TRN2 NeuronCores.

Problem: B=8, N=1024, C=1024, H=16, D=64, fp32 in/out.
Sharding: data-parallel over batch -- core b computes batch element b
end-to-end; no collectives.  ~267us HW (baseline 365us).

Design:
  - all matmuls bf16 (host casts x/w; rel err ~5.5e-3 < 2e-2 gate)
  - PE HAM clock management: dependency-free warm-up matmuls at t=0,
    then one continuous PE stream (prologue -> ACT-gated pair phase ->
    projection tail) so the clock stays at 2.4 GHz (the PE drops to
    1.2 GHz after any ~3.4us idle gap)
  - scores psum tiles [128,1024] hold BOTH heads of a slab in the free
    dim ([head-even n-half | head-odd n-half]) so one exp ACT releases
    both row-group matmuls together -> the two K=64 scores MMs run
    concurrently in disjoint PE row groups
  - middle phase: per (pair s, m-tile) slot emits 4 scores MMs + 2 exp
    ACTs + 4 AV MMs of pair s-1 (one slot later, so pair boundaries
    don't bubble the ACT queue) + one q/k/v slab-fill unit, keeping the
    ~147us of exp hidden under PE work
  - PSUM: scores 2x[128,1024] (4 banks) + AV acc 2x[65,512] (2) +
    fill/proj 2x[128,512] (2) = 8 banks exactly
  - AV: per (rowlo,nch) accumulate [65,512] over 8 m-tiles; row 64 is
    the softmax denominator via a ones column in the v slabs; divide =
    psum->SBUF copy (releases the bank early), DVE reciprocal, DRAM
    round-trip partition broadcast, DVE multiply; odd heads bounce
    through an SBUF->SBUF DMA to shift partitions 0-63 -> 64-127
  - tail: pair-7 nch0 AV runs inside pair 7 (filler psum banks) so its
    divide lands at phase end; proj n-rows 0-511 start immediately,
    overlapping the nch1 AV + divide; DMA loads are 2D slab transfers
    split across the sync/gpsimd queues, x strictly first
"""

import os
import sys

sys.path.insert(0, "/opt/trn_rl_repo")

import numpy as np

B, N, C = 8, 1024, 1024
H = 16
D = C // H  # 64
SCALE = D ** -0.5  # 0.125
P = 128
CT = C // P  # 8 contraction tiles of 128

_CACHE = {}

LAST_EXEC_NS = None


def _build():
    import concourse.bacc as bacc
    import concourse.tile as tile
    from concourse import mybir

    fp32 = mybir.dt.float32
    bf16 = mybir.dt.bfloat16
    AFT = mybir.ActivationFunctionType

    nc = bacc.Bacc(
        "TRN2",
        target_bir_lowering=False,
        debug=False,
        enable_asserts=False,
        num_devices=8,
    )
    xT = nc.dram_tensor("xT", [C, N], bf16, kind="ExternalInput")
    wqkvT = nc.dram_tensor("wqkvT", [C, 3 * C], bf16, kind="ExternalInput")
    wprojT = nc.dram_tensor("wprojT", [C, C], bf16, kind="ExternalInput")
    bproj = nc.dram_tensor("bproj", [C], fp32, kind="ExternalInput")
    y = nc.dram_tensor("y", [N, C], fp32, kind="ExternalOutput")

    tap = os.environ.get("MHSA_KERNEL_DEBUG_TAP", "")

    with tile.TileContext(nc) as tc:
        with (
            tc.tile_pool(name="consts", bufs=1) as consts,
            tc.tile_pool(name="xp", bufs=1) as xp,
            tc.tile_pool(name="wq", bufs=4) as wqp,
            tc.tile_pool(name="wp2", bufs=2) as wp2,
            tc.tile_pool(name="qk", bufs=16) as qkp,
            tc.tile_pool(name="vp", bufs=8) as vpp,
            tc.tile_pool(name="et", bufs=26) as etp,
            tc.tile_pool(name="pj", bufs=8) as pjp,
            tc.tile_pool(name="sm", bufs=2) as smp,
            tc.tile_pool(name="avs", bufs=3) as avsp,
            tc.tile_pool(name="rb", bufs=2) as rbp,
            tc.tile_pool(name="tm", bufs=1) as tmp_pool,
            tc.tile_pool(name="ot", bufs=3) as otp,
            tc.tile_pool(name="dscr", bufs=8, space="DRAM") as dscr,
            tc.tile_pool(name="ps_sc", bufs=2, space="PSUM") as ps_sc,
            tc.tile_pool(name="ps_av", bufs=2, space="PSUM") as ps_av,
            tc.tile_pool(name="ps_fl", bufs=2, space="PSUM") as ps_fl,
        ):
            # x resident as ONE tile [128, 8*1024]: column block ci holds
            # xT rows [128ci, 128ci+128) (one coalesced DMA, not 8)
            xs = xp.tile([P, CT * N], bf16, name="xs", tag="xs")

            def xsl(ci, lo, hi):
                return xs[:, ci * N + lo : ci * N + hi]
            qts = [qkp.tile([P, N], bf16, name=f"qs{i}", tag="qk") for i in range(CT)]
            kts = [qkp.tile([P, N], bf16, name=f"ks{i}", tag="qk") for i in range(CT)]
            vss = [
                vpp.tile([P, H * 65], bf16, name=f"vs{i}", tag="vs") for i in range(CT)
            ]
            vvs = [v[:].rearrange("p (h e) -> p h e", e=65) for v in vss]
            pjs = [pjp.tile([P, N], bf16, name=f"pj{i}", tag="pj") for i in range(CT)]
            bb = consts.tile([P, C], fp32)

            # ---- ACT table warm-up: tiny exp so the ~2.7us table load
            # happens during the prologue DMA, not at the first real exp.
            junk = smp.tile([1, 16], fp32, name="junk", tag="junk")
            junk2 = smp.tile([1, 16], fp32, name="junk2", tag="junk")
            nc.gpsimd.memset(junk[:], 0.0)
            nc.scalar.activation(junk2[:], junk[:], AFT.Exp, scale=1.0)

            # ---- PE HAM warm-up: ~5us of dependency-free junk matmuls at
            # t=0 so the PE clock is at 2.4 GHz (K=8/8) by the time the
            # DMA-fed prologue matmuls start.  Without this the whole
            # prologue runs at the cold 1.2 GHz clock.
            jw = consts.tile([P, 512], bf16, name="jw")
            nc.gpsimd.memset(jw[:], 0.0)
            jps = ps_fl.tile([1, 512], fp32, name="jfl", tag="fl")
            NJUNK = 30
            for i in range(NJUNK):
                # one accumulation group: MMs pipeline back-to-back with no
                # inter-MM semaphores; K=128 so HAM sees full-row activity
                nc.tensor.matmul(
                    jps[:],
                    lhsT=jw[:, 0:1],
                    rhs=jw[:, :],
                    start=(i == 0),
                    stop=(i == NJUNK - 1),
                )

            # ---- DMAs: plain 2D slab transfers (fast DMA path), spread
            # across the sync and gpsimd queues.  Priority order: x + the
            # q0/k0 weight blocks (prologue critical path), then the v
            # blocks (needed by pair-0 filler), then everything else.
            wtiles = {}

            def walloc():
                wt = wqp.tile([P, CT * 512], bf16, name="wob", tag="wob")
                return wt

            def load_w_chunk(wt, oblk, ci, eng):
                eng.dma_start(
                    wt[:, ci * 512 : (ci + 1) * 512],
                    wqkvT.ap()[
                        ci * P : (ci + 1) * P, oblk * 512 : (oblk + 1) * 512
                    ],
                )

            def load_w(oblk, flip=0):
                wt = walloc()
                for ci in range(CT):
                    eng = nc.sync if (ci + flip) % 2 == 0 else nc.gpsimd
                    load_w_chunk(wt, oblk, ci, eng)
                wtiles[oblk] = wt

            def wsl(oblk, ci, lo, hi):
                return wtiles[oblk][:, ci * 512 + lo : ci * 512 + hi]

            # v-first prologue: x + the v weight blocks have priority,
            # the q0/k0 blocks follow.  (oblk 4,5 buffers get reused by
            # oblk 1,3 once the v slabs are done -- allocation order
            # matters for the pool rotation.)
            wtiles[4] = walloc()
            wtiles[5] = walloc()
            # x strictly first on both queues (everything needs all of x)
            for ci in range(CT):
                eng = nc.sync if ci % 2 == 0 else nc.gpsimd
                eng.dma_start(
                    xs[:, ci * N : (ci + 1) * N],
                    xT.ap()[ci * P : (ci + 1) * P, :],
                )
            for ci in range(CT):
                load_w_chunk(wtiles[4], 4, ci, nc.gpsimd if ci % 2 == 0 else nc.sync)
            for ci in range(CT):
                load_w_chunk(wtiles[5], 5, ci, nc.sync if ci % 2 == 0 else nc.gpsimd)
            nc.gpsimd.dma_start(bb[:], bproj.ap().partition_broadcast(P))
            load_w(0)
            load_w(2, 1)

            # ---- slab emitters (each "unit" = half a slab: 8 MMs + 1 copy)
            def emit_qk_half(kind, s, half):
                oblk = (0 if kind == "q" else 2) + s // 4
                dst = (qts if kind == "q" else kts)[s]
                ps = ps_fl.tile([P, 512], fp32, name="fl", tag="fl")
                for ci in range(CT):
                    nc.tensor.matmul(
                        ps[:],
                        lhsT=wsl(oblk, ci, (s % 4) * P, (s % 4 + 1) * P),
                        rhs=xsl(ci, half * 512, (half + 1) * 512),
                        start=(ci == 0),
                        stop=(ci == CT - 1),
                    )
                nc.vector.tensor_copy(dst[:, half * 512 : (half + 1) * 512], ps[:])

            def emit_v_half(mi, vblk):
                if vblk == 0:
                    nc.gpsimd.memset(vvs[mi][:, :, 64:65], 1.0)
                ps = ps_fl.tile([P, 512], fp32, name="fl", tag="fl")
                for ci in range(CT):
                    nc.tensor.matmul(
                        ps[:],
                        lhsT=xsl(ci, mi * P, (mi + 1) * P),
                        rhs=wsl(4 + vblk, ci, 0, 512),
                        start=(ci == 0),
                        stop=(ci == CT - 1),
                    )
                nc.vector.tensor_copy(
                    vvs[mi][:, vblk * 8 : (vblk + 1) * 8, 0:64],
                    ps[:].rearrange("p (hh d) -> p hh d", d=64),
                )

            # ---- scores + exp for (pair s, m-tile mi)
            # psum tile layout: cols 0-511 = head 2s (rows 0:64 of slab),
            # cols 512-1023 = head 2s+1 -- both for one n-half.
            eta = [[None] * CT for _ in range(CT)]  # [s][mi] -> n 0:512
            etb = [[None] * CT for _ in range(CT)]  # [s][mi] -> n 512:1024

            def emit_scores(s, mi):
                ta = ps_sc.tile([P, N], fp32, name="sca", tag="sc")
                tb = ps_sc.tile([P, N], fp32, name="scb", tag="sc")
                for t, nch in ((ta, 0), (tb, 1)):
                    for rowlo in (0, 64):
                        nc.tensor.matmul(
                            t[:, (rowlo // 64) * 512 : (rowlo // 64) * 512 + 512],
                            lhsT=kts[s][rowlo : rowlo + 64, mi * P : (mi + 1) * P],
                            rhs=qts[s][
                                rowlo : rowlo + 64, nch * 512 : (nch + 1) * 512
                            ],
                            start=True,
                            stop=True,
                        )
                ea = etp.tile([P, N], bf16, name="eta", tag="et")
                eb = etp.tile([P, N], bf16, name="etb", tag="et")
                nc.scalar.activation(ea[:], ta[:], AFT.Exp, scale=SCALE)
                nc.scalar.activation(eb[:], tb[:], AFT.Exp, scale=SCALE)
                eta[s][mi] = ea
                etb[s][mi] = eb

            # ---- AV for pair sp, staged: nch0 over slots 0-3, nch1 4-7.
            accs = {}
            tail_divs = {(7, 1)}

            def emit_av_slot(sp, slot, acc_pool=None):
                nch = slot // 4
                j = slot % 4
                ets = eta if nch == 0 else etb
                if j == 0:
                    pool = acc_pool if acc_pool is not None else ps_av
                    tg = "sc" if acc_pool is not None else "av"
                    for rowlo in (0, 64):
                        accs[(sp, nch, rowlo)] = pool.tile(
                            [65, 512], fp32, name="av", tag=tg
                        )
                for mi_ in (2 * j, 2 * j + 1):
                    for rowlo in (0, 64):
                        nc.tensor.matmul(
                            accs[(sp, nch, rowlo)][:],
                            lhsT=vvs[mi_][:, 2 * sp + rowlo // 64, :],
                            rhs=ets[sp][mi_][
                                :, (rowlo // 64) * 512 : (rowlo // 64) * 512 + 512
                            ],
                            start=(mi_ == 0),
                            stop=(mi_ == 7),
                        )
                if j == 3:
                    emit_div(sp, nch, tail=(acc_pool is not None or (sp, nch) in tail_divs))

            def emit_av7_imm(mi_):
                # pair-7 nch0 AV accumulated inside pair 7 itself, in the
                # filler psum banks (no fills run during pair 7).  Its
                # divide then completes right at phase end, so proj mi0-3
                # can start without waiting for the whole AV(7) tail.
                if mi_ == 0:
                    for rowlo in (0, 64):
                        accs[(7, 0, rowlo)] = ps_fl.tile(
                            [65, 512], fp32, name="av7", tag="fl"
                        )
                for rowlo in (0, 64):
                    nc.tensor.matmul(
                        accs[(7, 0, rowlo)][:],
                        lhsT=vvs[mi_][:, 14 + rowlo // 64, :],
                        rhs=eta[7][mi_][
                            :, (rowlo // 64) * 512 : (rowlo // 64) * 512 + 512
                        ],
                        start=(mi_ == 0),
                        stop=(mi_ == 7),
                    )

            def emit_div(sp, nch, tail=False):
                # copy acc psum -> SBUF first: releases the psum bank after
                # ~0.7us instead of holding it through the whole divide
                # chain (the next AV group's MMs wait on that bank).  The
                # chain is queue-local: DVE (copy+recip+scr-DMA) then
                # gpsimd (broadcast-DMA+mul+shift-DMA) -- one cross-queue
                # hop total.  Tail divides use the then-idle ACT engine
                # for the copy.
                for rowlo in (0, 64):
                    acc = accs.pop((sp, nch, rowlo))
                    av = avsp.tile([65, 512], fp32, name="avc", tag="avc")
                    if tail and rowlo == 64:
                        nc.scalar.copy(av[:], acc[:])
                    else:
                        nc.vector.tensor_copy(av[:], acc[:])
                    # NB: reciprocal_approx_fast must start at partition 0
                    # and write a separate tile (partial-partition slices
                    # and in-place both produce garbage)
                    rcp = smp.tile([65, 512], fp32, name="rcp", tag="rcp")
                    nc.vector.reciprocal_approx_fast(rcp[:], av[:])
                    scr = dscr.tile([1, 512], fp32, name="scr", tag="scr")
                    # tail divides split their DMA hops across queues so the
                    # two rowlo chains run in parallel (sync is weight/y-only
                    # by then; mid-phase it would head-of-line block)
                    deng = nc.sync if (tail and rowlo == 64) else nc.gpsimd
                    deng.dma_start(scr[:], rcp[64:65, :])
                    rb = rbp.tile([64, 512], fp32, name="rb", tag="rb")
                    deng.dma_start(rb[:], scr[0, :].partition_broadcast(64))
                    dst = pjs[sp][
                        rowlo : rowlo + 64, nch * 512 : (nch + 1) * 512
                    ]
                    if rowlo == 0:
                        nc.vector.tensor_mul(dst, av[0:64, :], rb[:])
                    else:
                        tmp = tmp_pool.tile([64, 512], bf16, name="tmp", tag="tmp")
                        meng = nc.gpsimd if tail else nc.vector
                        meng.tensor_mul(tmp[:], av[0:64, :], rb[:])
                        deng2 = nc.sync if tail else nc.gpsimd
                        deng2.dma_start(dst, tmp[:])

            # ---- prologue PE: v0-3 (chasing the x/w DMA), q0,k0,q1,k1;
            # v4-7 go as pair-0 filler (AV(0) during pair 1 needs them).
            run_heads = tap in ("", "pj", "et")
            if run_heads:
                for mi in range(4):
                    for vblk in range(2):
                        emit_v_half(mi, vblk)
                for s in (0, 1):
                    for half in range(2):
                        emit_qk_half("q", s, half)
                    for half in range(2):
                        emit_qk_half("k", s, half)

                pwts = []

                def load_wproj():
                    for och in range(2):
                        wt = wp2.tile([P, CT * 512], bf16, name="pwt", tag="pwt")
                        for ci in range(CT):
                            nc.sync.dma_start(
                                wt[:, ci * 512 : (ci + 1) * 512],
                                wprojT.ap()[
                                    ci * P : (ci + 1) * P,
                                    och * 512 : (och + 1) * 512,
                                ],
                            )
                        pwts.append(wt)

                # filler units per pair, consumed one per slot until the
                # pair's list is empty.  Deadlines: v*.vblk0 before pair 1
                # (AV(0) heads 0/1), (k,q)(s) before pair s-1 ends,
                # v*.vblk1 before pair 4 ends (AV(4) heads 8/9).
                pair_units = [[] for _ in range(8)]
                pair_units[0] = [("v", mi, vblk) for mi in range(4, 8) for vblk in (0, 1)]
                for s in range(1, 7):
                    pair_units[s] = [("k", s + 1, 0), ("k", s + 1, 1),
                                     ("q", s + 1, 0), ("q", s + 1, 1)]

                def pop_unit(s):
                    if pair_units[s]:
                        kind, a, b = pair_units[s].pop(0)
                        if kind == "v":
                            emit_v_half(a, b)
                        else:
                            emit_qk_half(kind, a, b)

                # ---- main ACT-gated phase.  AV for pair s-1 lags one slot
                # behind the scores of pair s so the next pair's scores MMs
                # always precede the AV group that waits on the previous
                # pair's final exp (kills the pair-boundary ACT bubble).
                for s in range(CT):
                    for mi in range(CT):
                        emit_scores(s, mi)
                        if s == 7:
                            # pair 6's exps are long done: run its AV
                            # unshifted so div(6,*) completes in-phase
                            if mi == 0:
                                emit_av_slot(5, 7)
                            emit_av_slot(6, mi)
                        elif s >= 1 and mi >= 1:
                            emit_av_slot(s - 1, mi - 1)
                        elif s >= 2 and mi == 0:
                            emit_av_slot(s - 2, 7)
                        if s == 7 and mi >= 2:
                            emit_av7_imm(mi - 2)
                        # filler budget: pair 0 has no AV -> 1 unit/slot;
                        # later pairs 1 unit every other slot
                        pop_unit(s)
                        # late weight loads: oblk1/3 reuse the oblk4/5
                        # buffers, so emit only after the last v-slab
                        # filler MMs (end of pair 0) have been emitted.
                        if s == 1 and mi == 0:
                            load_w(1)
                            load_w(3, 1)
                            load_wproj()
                for s in range(8):
                    while pair_units[s]:
                        pop_unit(s)

                # ---- tail: AV(7) + divides interleaved with projection
                def emit_proj(mi):
                    for och in range(2):
                        ps = ps_fl.tile([P, 512], fp32, name="fl", tag="fl")
                        for ci in range(CT):
                            nc.tensor.matmul(
                                ps[:],
                                lhsT=pjs[ci][:, mi * P : (mi + 1) * P],
                                rhs=pwts[och][:, ci * 512 : (ci + 1) * 512],
                                start=(ci == 0),
                                stop=(ci == CT - 1),
                            )
                        ot = otp.tile([P, 512], fp32, name="ot", tag="ot")
                        nc.vector.tensor_add(
                            ot[:], ps[:], bb[:, och * 512 : (och + 1) * 512]
                        )
                        yeng = nc.sync if (2 * mi + och) % 2 == 0 else nc.scalar
                        yeng.dma_start(
                            y.ap()[mi * P : (mi + 1) * P, och * 512 : (och + 1) * 512],
                            ot[:],
                        )

                # tail: finish the shifted AV(6) + pair-7 AV, overlapping
                # divides and projection.
                emit_av7_imm(6)
                emit_av7_imm(7)
                emit_div(7, 0, tail=True)
                if tap == "":
                    for mi in range(4):
                        emit_proj(mi)        # gated only on nch0 divides
                for slot in range(4, 8):
                    emit_av_slot(7, slot)    # nch1 + div(7,1)
                if tap == "":
                    for mi in range(4, 8):
                        emit_proj(mi)
            else:
                # debug taps for q/k/v only: emit all slabs plainly
                for mi in range(CT):
                    for vblk in range(2):
                        emit_v_half(mi, vblk)
                for s in range(CT):
                    for kind in ("q", "k"):
                        if s >= 4 and (0 if kind == "q" else 2) + 1 not in wtiles:
                            pass
                        for half in range(2):
                            if s == 4 and half == 0 and kind == "q":
                                load_w(1)
                                load_w(3, 1)
                            emit_qk_half(kind, s, half)

            # ---- debug taps
            if tap in ("q", "k"):
                slabs = qts if tap == "q" else kts
                for s in range(CT):
                    ct = otp.tile([P, N], fp32, name="dbgt", tag="dbgt")
                    nc.vector.tensor_copy(ct[:], slabs[s][:])
                    nc.sync.dma_start(y.ap()[s * P : (s + 1) * P, :], ct[:])
            elif tap == "v":
                for mi in range(CT):
                    ct = otp.tile([P, N], fp32, name="dbgt", tag="dbgt")
                    nc.vector.tensor_copy(
                        ct[:].rearrange("p (h d) -> p h d", d=64),
                        vvs[mi][:, :, 0:64],
                    )
                    nc.sync.dma_start(y.ap()[mi * P : (mi + 1) * P, :], ct[:])
            elif tap == "et":
                # dump pair 7's eta tiles (bf16 -> fp32)
                for mi in range(CT):
                    ct = otp.tile([P, N], fp32, name="dbgt", tag="dbgt")
                    nc.vector.tensor_copy(ct[:], eta[7][mi][:])
                    nc.sync.dma_start(y.ap()[mi * P : (mi + 1) * P, :], ct[:])
            elif tap == "pj":
                for s in range(CT):
                    ct = otp.tile([P, N], fp32, name="dbgt", tag="dbgt")
                    nc.vector.tensor_copy(ct[:], pjs[s][:])
                    nc.sync.dma_start(y.ap()[s * P : (s + 1) * P, :], ct[:])

    nc.compile()
    return nc


def kernel(x, w_qkv, w_proj, b_proj):
    global LAST_EXEC_NS
    import ml_dtypes
    from concourse.bass_utils import run_bass_kernel_spmd

    bf = ml_dtypes.bfloat16
    x = np.asarray(x, dtype=np.float32)
    w_qkv = np.asarray(w_qkv, dtype=np.float32)
    w_proj = np.asarray(w_proj, dtype=np.float32)
    b_proj = np.asarray(b_proj, dtype=np.float32)

    if "nc" not in _CACHE:
        _CACHE["nc"] = _build()
    nc = _CACHE["nc"]

    wqkvT = np.ascontiguousarray(w_qkv.astype(bf).T)
    wprojT = np.ascontiguousarray(w_proj.astype(bf).T)
    xb = x.astype(bf)
    in_maps = [
        {
            "xT": np.ascontiguousarray(xb[b].T),
            "wqkvT": wqkvT,
            "wprojT": wprojT,
            "bproj": b_proj,
        }
        for b in range(B)
    ]
    res = run_bass_kernel_spmd(nc, in_maps, core_ids=list(range(B)))
    if res.exec_time_ns is not None:
        LAST_EXEC_NS = res.exec_time_ns
    return np.stack([res.results[b]["y"] for b in range(B)], axis=0)
